# revision 14
# baseline (speedup 1.0000x reference)
"""Self-contained Trainium2 Bass kernel for nn_DDGPT_17248588661280.

The reference model's 4 "layers" are fully independent (each uses a fresh
embedding of its own token tensor), so the 8 cores each run one
(layer, batch) unit end-to-end with no collectives:
    core c -> layer c//2, batch c%2.
Each core computes: x = wte[tok] + wpe -> transformer block -> logits = x@wte.T
-> per-token logsumexp + target logit -> loss partial. Host sums loss partials
and takes cores 6,7 logits (layer 3) as the output logits.

Device layout: activations transposed [feature, token]; all matmuls bf16
except QK attention scores which use a hi+lo bf16 decomposition (3 matmuls)
for ~f32 precision (scores are scaled by sqrt(64)=8, exponent-sensitive).
"""
import numpy as np
import ml_dtypes

import concourse.bass as bass
import concourse.tile as tile
from concourse import bacc, mybir
from concourse.bass_utils import run_bass_kernel_spmd

F32 = mybir.dt.float32
BF16 = mybir.dt.bfloat16
AF = mybir.ActivationFunctionType
ALU = mybir.AluOpType

B, T, V, E, H, L = 2, 1024, 32000, 768, 12, 4
HD = E // H          # 64
EC = E // 128        # 6 e-chunks
MC3 = 3 * E // 128   # 18 chunks of 3E
NQ = 2               # token chunks of 512
TQ = 512
NV = 63              # lm_head vocab chunks: 62x512 + 1x256
EXP_BIAS = -88.0     # constant subtracted inside attention exp (overflow guard)

TRACE = False        # test.py sets True to get exec_time_ns
DEBUG = False
LAST_RESULT = {}

_BUILD_CACHE = {}
_SHARED = {}


def _build():
    key = (TRACE, DEBUG)
    if key in _BUILD_CACHE:
        return _BUILD_CACHE[key]
    nc = bacc.Bacc(trn_type="TRN2", target_bir_lowering=False, debug=False)

    par = {}
    def dp(name, shape, dt, out=False):
        par[name] = nc.declare_dram_parameter(name, shape, dt, isOutput=out)

    dp("xembT", [E, T], F32)
    dp("tgtembT", [E, T], F32)
    dp("wpeT", [E, T], F32)
    for nm in ["ln1w", "ln1b", "ln2w", "ln2b"]:
        dp(nm, [128, EC], F32)
    dp("attn_w", [E, 3 * E], BF16); dp("attn_b", [1, 3 * E], BF16)
    dp("attn_wqk_lo", [E, 2 * E], BF16)
    dp("proj_w", [E, E], BF16); dp("proj_b", [1, E], BF16)
    dp("fc1_w", [E, 3 * E], BF16); dp("fc1_b", [1, 3 * E], BF16)
    dp("fc2_w", [3 * E, E], BF16); dp("fc2_b", [1, E], BF16)
    dp("wteT", [E, V], BF16)
    dp("logits", [T, V], F32, out=True)
    dp("lossv", [1, 2], F32, out=True)
    if DEBUG:
        dp("d_xT", [E, T], F32, out=True); dp("d_h1", [E, T], BF16, out=True)
        dp("d_qhi", [E, T], BF16, out=True); dp("d_khi", [E, T], BF16, out=True)
        dp("d_v", [T, E], BF16, out=True); dp("d_yT", [E, T], BF16, out=True)
        dp("d_x2", [E, T], F32, out=True); dp("d_g", [3 * E, T], BF16, out=True)
        dp("d_xout", [E, T], F32, out=True); dp("d_tl", [1, T], F32, out=True)
        dp("d_lse", [128, 8], F32, out=True)
        dp("d_qlo", [E, T], BF16, out=True); dp("d_klo", [E, T], BF16, out=True)
        dp("d_ex0", [T, TQ], BF16, out=True); dp("d_dn0", [1, T], F32, out=True)
        dp("d_qf32", [E, T], F32, out=True); dp("d_h1lo", [E, T], BF16, out=True)
        dp("d_yraw0", [128, T], F32, out=True)

    with tile.TileContext(nc) as tc:
        _emit(nc, tc, par)

    nc.compile()
    _BUILD_CACHE[key] = nc
    return nc


def _emit(nc, tc, par):
    from contextlib import ExitStack
    ctx = ExitStack()
    with ctx:
        consts = ctx.enter_context(tc.tile_pool(name="consts", bufs=1))
        ones_col_bf = consts.tile([128, 1], BF16)
        nc.vector.memset(ones_col_bf[:], 1.0)
        ones_row_bf = consts.tile([1, TQ], BF16)
        nc.vector.memset(ones_row_bf[:], 1.0)
        ones_col_f32 = consts.tile([128, 1], F32)
        nc.vector.memset(ones_col_f32[:], 1.0)
        ones_row_f32 = consts.tile([1, 128], F32)
        nc.vector.memset(ones_row_f32[:], 1.0)
        neg88 = consts.tile([128, 1], F32)
        nc.vector.memset(neg88[:], EXP_BIAS)

        lncol = ctx.enter_context(tc.tile_pool(name="lncol", bufs=1))
        ln_tiles = {}
        for nm in ["ln1w", "ln1b", "ln2w", "ln2b"]:
            t = lncol.tile([128, EC], F32, tag=nm)
            nc.sync.dma_start(t[:], par[nm].ap())
            ln_tiles[nm] = t

        brow = ctx.enter_context(tc.tile_pool(name="brow", bufs=1))
        b_tiles = {}
        for nm, w in [("attn_b", 3 * E), ("proj_b", E), ("fc1_b", 3 * E), ("fc2_b", E)]:
            t = brow.tile([1, w], BF16, tag=nm)
            nc.sync.dma_start(t[:], par[nm].ap())
            b_tiles[nm] = t

        # persistent activations: xT doubles as x2/xout via in-place residual adds
        xpool = ctx.enter_context(tc.tile_pool(name="xpool", bufs=1))
        xT = [xpool.tile([128, T], F32, tag=f"xT{t}", name=f"xT{t}") for t in range(EC)]
        hpool = ctx.enter_context(tc.tile_pool(name="hpool", bufs=1))
        losspool = ctx.enter_context(tc.tile_pool(name="losspool", bufs=1))
        tl_sb = losspool.tile([1, T], F32, tag="tlsb")

        # ---------- embed: xT = xembT + wpeT ----------
        with tc.tile_pool(name="emb", bufs=3) as emb:
            for t in range(EC):
                xe = emb.tile([128, T], F32, tag="xe")
                nc.sync.dma_start(xe[:], par["xembT"].ap()[t * 128:(t + 1) * 128, :])
                wp = emb.tile([128, T], F32, tag="wp")
                nc.sync.dma_start(wp[:], par["wpeT"].ap()[t * 128:(t + 1) * 128, :])
                nc.vector.tensor_tensor(out=xT[t][:], in0=xe[:], in1=wp[:], op=ALU.add)
        if DEBUG:
            for t in range(EC):
                nc.sync.dma_start(par["d_xT"].ap()[t * 128:(t + 1) * 128, :], xT[t][:])

        # ---------- layernorm helper (transposed layout; stats via ones-matmul) ----
        def layernorm(x_tiles, wcol, bcol, tag, lo_pool=None):
            h_bf = [hpool.tile([128, T], BF16, tag=f"lnh{t}", name=f"{tag}h{t}") for t in range(EC)]
            h_lo = None
            if lo_pool is not None:
                h_lo = [lo_pool.tile([128, T], BF16, tag=f"hlo{t}", name=f"{tag}hlo{t}")
                        for t in range(EC)]
            with tc.tile_pool(name=f"{tag}tmp", bufs=1) as tp, \
                 tc.tile_pool(name=f"{tag}st", bufs=1) as stp, \
                 tc.tile_pool(name=f"{tag}ps", bufs=2, space="PSUM") as psp, \
                 tc.tile_pool(name=f"{tag}bc", bufs=2, space="PSUM") as bcp:
                xbf = [tp.tile([128, T], BF16, tag=f"xbf{t}", name=f"{tag}xbf{t}") for t in range(EC)]
                x2bf = [tp.tile([128, T], BF16, tag=f"x2bf{t}", name=f"{tag}x2bf{t}") for t in range(EC)]
                for t in range(EC):
                    nc.vector.tensor_copy(out=xbf[t][:], in_=x_tiles[t][:])
                    nc.vector.tensor_tensor(out=x2bf[t][:], in0=xbf[t][:], in1=xbf[t][:], op=ALU.mult)
                mu = stp.tile([1, T], F32, tag="mu")
                ssn = stp.tile([1, T], F32, tag="ssn")
                for q in range(NQ):
                    qs = slice(q * TQ, (q + 1) * TQ)
                    mps = psp.tile([1, TQ], F32, tag="mps")
                    sps = psp.tile([1, TQ], F32, tag="sps")
                    for t in range(EC):
                        nc.tensor.matmul(mps[:], ones_col_bf[:], xbf[t][:, qs],
                                         start=(t == 0), stop=(t == EC - 1))
                    for t in range(EC):
                        nc.tensor.matmul(sps[:], ones_col_bf[:], x2bf[t][:, qs],
                                         start=(t == 0), stop=(t == EC - 1))
                    nc.vector.tensor_scalar(out=mu[:, qs], in0=mps[:],
                                            scalar1=1.0 / E, scalar2=None, op0=ALU.mult)
                    nc.vector.tensor_scalar(out=ssn[:, qs], in0=sps[:],
                                            scalar1=1.0 / E, scalar2=1e-5, op0=ALU.mult, op1=ALU.add)
                # var+eps = ssn - mu^2 (in place); rstd = 1/sqrt(var+eps) + Newton step
                ta = stp.tile([1, T], F32, tag="ta")
                tb = stp.tile([1, T], F32, tag="tb")
                nc.vector.tensor_tensor(out=ta[:], in0=mu[:], in1=mu[:], op=ALU.mult)
                nc.vector.tensor_tensor(out=ssn[:], in0=ssn[:], in1=ta[:], op=ALU.subtract)
                nc.scalar.activation(ta[:], ssn[:], AF.Sqrt, bias=0.0, scale=1.0)
                nc.vector.reciprocal(out=tb[:], in_=ta[:])
                nc.vector.tensor_tensor(out=ta[:], in0=tb[:], in1=tb[:], op=ALU.mult)
                nc.vector.tensor_tensor(out=ta[:], in0=ta[:], in1=ssn[:], op=ALU.mult)
                nc.vector.tensor_scalar(out=ta[:], in0=ta[:], scalar1=-0.5, scalar2=1.5,
                                        op0=ALU.mult, op1=ALU.add)
                nc.vector.tensor_tensor(out=tb[:], in0=tb[:], in1=ta[:], op=ALU.mult)
                for q in range(NQ):
                    qs = slice(q * TQ, (q + 1) * TQ)
                    mub = bcp.tile([128, TQ], F32, tag="mub")
                    nc.tensor.matmul(mub[:], ones_row_f32[:], mu[:, qs], start=True, stop=True)
                    rsb = bcp.tile([128, TQ], F32, tag="rsb")
                    nc.tensor.matmul(rsb[:], ones_row_f32[:], tb[:, qs], start=True, stop=True)
                    for t in range(EC):
                        tt = tp.tile([128, TQ], F32, tag="lt")
                        nc.vector.tensor_tensor(out=tt[:], in0=x_tiles[t][:, qs], in1=mub[:],
                                                op=ALU.subtract)
                        nc.vector.tensor_tensor(out=tt[:], in0=tt[:], in1=rsb[:], op=ALU.mult)
                        if h_lo is None:
                            nc.vector.tensor_scalar(out=h_bf[t][:, qs], in0=tt[:],
                                                    scalar1=wcol[:, t:t + 1], scalar2=bcol[:, t:t + 1],
                                                    op0=ALU.mult, op1=ALU.add)
                        else:
                            nc.vector.tensor_scalar(out=tt[:], in0=tt[:],
                                                    scalar1=wcol[:, t:t + 1], scalar2=bcol[:, t:t + 1],
                                                    op0=ALU.mult, op1=ALU.add)
                            nc.scalar.copy(out=h_bf[t][:, qs], in_=tt[:])
                            nc.vector.tensor_tensor(out=h_lo[t][:, qs], in0=tt[:],
                                                    in1=h_bf[t][:, qs], op=ALU.subtract)
            return (h_bf, h_lo) if lo_pool is not None else h_bf

        # ---------- LN1 + QKV + attention + proj (scoped so q/k/v free early) ----
        ypool = ctx.enter_context(tc.tile_pool(name="ypool", bufs=1))

        with tc.tile_pool(name="qkpool", bufs=1) as qkpool:
            h1, h1_lo = layernorm(xT, ln_tiles["ln1w"], ln_tiles["ln1b"], "ln1", lo_pool=qkpool)
            if DEBUG:
                for t in range(EC):
                    nc.sync.dma_start(par["d_h1"].ap()[t * 128:(t + 1) * 128, :], h1[t][:])
            q_hi = [qkpool.tile([128, T], BF16, tag=f"qhi{t}", name=f"qhi{t}") for t in range(EC)]
            q_lo = [qkpool.tile([128, T], BF16, tag=f"qlo{t}", name=f"qlo{t}") for t in range(EC)]
            k_hi = [qkpool.tile([128, T], BF16, tag=f"khi{t}", name=f"khi{t}") for t in range(EC)]
            k_lo = [qkpool.tile([128, T], BF16, tag=f"klo{t}", name=f"klo{t}") for t in range(EC)]
            v_sb = [qkpool.tile([128, E], BF16, tag=f"vsb{t}", name=f"vsb{t}") for t in range(8)]

            with tc.tile_pool(name="awpool", bufs=1) as awp, \
                 tc.tile_pool(name="qkps", bufs=3, space="PSUM") as qkps:
                aw = [awp.tile([128, 3 * E], BF16, tag=f"aw{t}", name=f"aw{t}") for t in range(EC)]
                awlo = [awp.tile([128, 2 * E], BF16, tag=f"awlo{t}", name=f"awlo{t}") for t in range(EC)]
                for t in range(EC):
                    nc.sync.dma_start(aw[t][:], par["attn_w"].ap()[t * 128:(t + 1) * 128, :])
                    nc.sync.dma_start(awlo[t][:], par["attn_wqk_lo"].ap()[t * 128:(t + 1) * 128, :])
                for m in range(12):
                    hi_t, lo_t = (q_hi, q_lo) if m < EC else (k_hi, k_lo)
                    mt = m % EC
                    for q in range(NQ):
                        qs = slice(q * TQ, (q + 1) * TQ)
                        ps = qkps.tile([128, TQ], F32, tag="qk")
                        ms = slice(m * 128, (m + 1) * 128)
                        for t in range(EC):
                            nc.tensor.matmul(ps[:], aw[t][:, ms], h1[t][:, qs],
                                             start=(t == 0), stop=False)
                            nc.tensor.matmul(ps[:], aw[t][:, ms], h1_lo[t][:, qs],
                                             start=False, stop=False)
                            nc.tensor.matmul(ps[:], awlo[t][:, ms], h1[t][:, qs],
                                             start=False, stop=False)
                        nc.tensor.matmul(ps[:], b_tiles["attn_b"][:, m * 128:(m + 1) * 128],
                                         ones_row_bf[:], start=False, stop=True)
                        if DEBUG and m < EC:
                            qf = awp.tile([128, TQ], F32, tag="qfdbg", bufs=2)
                            nc.vector.tensor_copy(out=qf[:], in_=ps[:])
                            nc.sync.dma_start(par["d_qf32"].ap()[mt * 128:(mt + 1) * 128, qs], qf[:])
                        nc.scalar.copy(out=hi_t[mt][:, qs], in_=ps[:])
                        nc.vector.tensor_tensor(out=lo_t[mt][:, qs], in0=ps[:], in1=hi_t[mt][:, qs],
                                                op=ALU.subtract)
                for tok in range(8):
                    ts_ = slice(tok * 128, (tok + 1) * 128)
                    for half in range(2):
                        ps = qkps.tile([128, 384], F32, tag="vp")
                        c0 = 2 * E + half * 384
                        for t in range(EC):
                            nc.tensor.matmul(ps[:], h1[t][:, ts_], aw[t][:, c0:c0 + 384],
                                             start=(t == 0), stop=False)
                        nc.tensor.matmul(ps[:], ones_row_bf[:, 0:128], b_tiles["attn_b"][:, c0:c0 + 384],
                                         start=False, stop=True)
                        nc.vector.tensor_copy(out=v_sb[tok][:, half * 384:(half + 1) * 384], in_=ps[:])
            if DEBUG:
                for t in range(EC):
                    nc.sync.dma_start(par["d_h1lo"].ap()[t * 128:(t + 1) * 128, :], h1_lo[t][:])
                    nc.sync.dma_start(par["d_qhi"].ap()[t * 128:(t + 1) * 128, :], q_hi[t][:])
                    nc.sync.dma_start(par["d_khi"].ap()[t * 128:(t + 1) * 128, :], k_hi[t][:])
                    nc.sync.dma_start(par["d_qlo"].ap()[t * 128:(t + 1) * 128, :], q_lo[t][:])
                    nc.sync.dma_start(par["d_klo"].ap()[t * 128:(t + 1) * 128, :], k_lo[t][:])
                for tok in range(8):
                    nc.sync.dma_start(par["d_v"].ap()[tok * 128:(tok + 1) * 128, :], v_sb[tok][:])

            # attention: ST = (K^T Q)*8 in hi-lo bf16; P = exp(ST-88); y = V^T P / den
            yT = [ypool.tile([128, T], BF16, tag=f"yT{t}", name=f"yT{t}") for t in range(EC)]
            with tc.tile_pool(name="stps", bufs=2, space="PSUM") as stps, \
                 tc.tile_pool(name="avps", bufs=2, space="PSUM") as avps, \
                 tc.tile_pool(name="denps", bufs=2, space="PSUM") as denps, \
                 tc.tile_pool(name="bcps", bufs=1, space="PSUM") as bcps, \
                 tc.tile_pool(name="expsb", bufs=4) as expsb, \
                 tc.tile_pool(name="attmisc", bufs=4) as attmisc:
                for q in range(NQ):
                    qs = slice(q * TQ, (q + 1) * TQ)
                    for t in range(EC):
                        av = avps.tile([128, TQ], F32, tag="av")
                        rden = [None, None]
                        for sub in range(2):
                            h = 2 * t + sub
                            rs = slice(sub * 64, sub * 64 + 64)
                            den = denps.tile([1, TQ], F32, tag="den")
                            for kc in range(8):
                                ks = slice(kc * 128, (kc + 1) * 128)
                                st = stps.tile([128, TQ], F32, tag="st")
                                nc.tensor.matmul(st[:], k_hi[t][rs, ks], q_hi[t][rs, qs], start=True, stop=False)
                                nc.tensor.matmul(st[:], k_lo[t][rs, ks], q_hi[t][rs, qs], start=False, stop=False)
                                nc.tensor.matmul(st[:], k_hi[t][rs, ks], q_lo[t][rs, qs], start=False, stop=True)
                                ex = expsb.tile([128, TQ], BF16, tag="ex")
                                nc.scalar.activation(ex[:], st[:], AF.Exp, bias=neg88[:], scale=8.0)
                                if DEBUG and h == 0 and q == 0:
                                    nc.sync.dma_start(par["d_ex0"].ap()[kc * 128:(kc + 1) * 128, :], ex[:])
                                nc.tensor.matmul(av[rs, :], v_sb[kc][:, h * HD:(h + 1) * HD], ex[:],
                                                 start=(kc == 0), stop=(kc == 7))
                                nc.tensor.matmul(den[:], ones_col_bf[:], ex[:],
                                                 start=(kc == 0), stop=(kc == 7))
                            if DEBUG and h == 0:
                                dcp = attmisc.tile([1, TQ], F32, tag="dcp")
                                nc.vector.tensor_copy(out=dcp[:], in_=den[:])
                                nc.sync.dma_start(par["d_dn0"].ap()[:, qs], dcp[:])
                            rd = attmisc.tile([1, TQ], BF16, tag="rd")
                            with nc.allow_low_precision(reason="bf16 rden: 0.4% on attn weights is within budget"):
                                nc.vector.reciprocal(out=rd[:], in_=den[:])
                            rden[sub] = rd
                        bc = bcps.tile([128, TQ], F32, tag="bc")
                        nc.tensor.matmul(bc[0:64, :], ones_row_bf[:, 0:64], rden[0][:], start=True, stop=True)
                        nc.tensor.matmul(bc[64:128, :], ones_row_bf[:, 0:64], rden[1][:], start=True, stop=True)
                        bcs = attmisc.tile([128, TQ], BF16, tag="bcs")
                        nc.vector.tensor_copy(out=bcs[:], in_=bc[:])
                        if DEBUG and t == 0:
                            ycp = attmisc.tile([128, TQ], F32, tag="ycp")
                            nc.vector.tensor_copy(out=ycp[:], in_=av[:])
                            nc.sync.dma_start(par["d_yraw0"].ap()[:, qs], ycp[:])
                        nc.vector.tensor_tensor(out=yT[t][:, qs], in0=av[:], in1=bcs[:], op=ALU.mult)
        if DEBUG:
            for t in range(EC):
                nc.sync.dma_start(par["d_yT"].ap()[t * 128:(t + 1) * 128, :], yT[t][:])

        # ---------- proj + residual (in-place into xT) ----------
        with tc.tile_pool(name="pwpool", bufs=1) as pwp, \
             tc.tile_pool(name="pps", bufs=3, space="PSUM") as pps:
            pw = [pwp.tile([128, E], BF16, tag=f"pw{t}", name=f"pw{t}") for t in range(EC)]
            for t in range(EC):
                nc.sync.dma_start(pw[t][:], par["proj_w"].ap()[t * 128:(t + 1) * 128, :])
            for m in range(EC):
                for q in range(NQ):
                    qs = slice(q * TQ, (q + 1) * TQ)
                    ps = pps.tile([128, TQ], F32, tag="pp")
                    for t in range(EC):
                        nc.tensor.matmul(ps[:], pw[t][:, m * 128:(m + 1) * 128], yT[t][:, qs],
                                         start=(t == 0), stop=False)
                    nc.tensor.matmul(ps[:], b_tiles["proj_b"][:, m * 128:(m + 1) * 128],
                                     ones_row_bf[:], start=False, stop=True)
                    nc.vector.tensor_tensor(out=xT[m][:, qs], in0=xT[m][:, qs], in1=ps[:], op=ALU.add)
        if DEBUG:
            for t in range(EC):
                nc.sync.dma_start(par["d_x2"].ap()[t * 128:(t + 1) * 128, :], xT[t][:])

        # ---------- LN2 + fc1 + gelu ----------
        h2 = layernorm(xT, ln_tiles["ln2w"], ln_tiles["ln2b"], "ln2")
        with tc.tile_pool(name="gpool", bufs=1) as gpool:
            g_bf = [gpool.tile([128, T], BF16, tag=f"g{t}", name=f"g{t}") for t in range(MC3)]
            with tc.tile_pool(name="f1pool", bufs=1) as f1p, \
                 tc.tile_pool(name="f1ps", bufs=3, space="PSUM") as f1ps:
                f1 = [f1p.tile([128, 3 * E], BF16, tag=f"f1{t}", name=f"f1{t}") for t in range(EC)]
                for t in range(EC):
                    nc.sync.dma_start(f1[t][:], par["fc1_w"].ap()[t * 128:(t + 1) * 128, :])
                for m in range(MC3):
                    for q in range(NQ):
                        qs = slice(q * TQ, (q + 1) * TQ)
                        ps = f1ps.tile([128, TQ], F32, tag="f1p")
                        for t in range(EC):
                            nc.tensor.matmul(ps[:], f1[t][:, m * 128:(m + 1) * 128], h2[t][:, qs],
                                             start=(t == 0), stop=False)
                        nc.tensor.matmul(ps[:], b_tiles["fc1_b"][:, m * 128:(m + 1) * 128],
                                         ones_row_bf[:], start=False, stop=True)
                        nc.scalar.activation(g_bf[m][:, qs], ps[:], AF.Gelu_apprx_tanh, bias=0.0, scale=1.0)
            if DEBUG:
                for m in range(MC3):
                    nc.sync.dma_start(par["d_g"].ap()[m * 128:(m + 1) * 128, :], g_bf[m][:])

            # ---------- fc2 + residual (in-place into xT) + bf16 copy ----------
            xout_bf = [xpool.tile([128, T], BF16, tag=f"xoutbf{t}", name=f"xoutbf{t}") for t in range(EC)]
            with tc.tile_pool(name="f2pool", bufs=1) as f2p, \
                 tc.tile_pool(name="f2ps", bufs=3, space="PSUM") as f2ps:
                f2 = [f2p.tile([128, E], BF16, tag=f"f2{t}", name=f"f2{t}") for t in range(MC3)]
                for t in range(MC3):
                    nc.sync.dma_start(f2[t][:], par["fc2_w"].ap()[t * 128:(t + 1) * 128, :])
                for m in range(EC):
                    for q in range(NQ):
                        qs = slice(q * TQ, (q + 1) * TQ)
                        ps = f2ps.tile([128, TQ], F32, tag="f2p")
                        for t in range(MC3):
                            nc.tensor.matmul(ps[:], f2[t][:, m * 128:(m + 1) * 128], g_bf[t][:, qs],
                                             start=(t == 0), stop=False)
                        nc.tensor.matmul(ps[:], b_tiles["fc2_b"][:, m * 128:(m + 1) * 128],
                                         ones_row_bf[:], start=False, stop=True)
                        nc.vector.tensor_tensor(out=xT[m][:, qs], in0=xT[m][:, qs], in1=ps[:], op=ALU.add)
                        nc.vector.tensor_copy(out=xout_bf[m][:, qs], in_=xT[m][:, qs])
        if DEBUG:
            for t in range(EC):
                nc.sync.dma_start(par["d_xout"].ap()[t * 128:(t + 1) * 128, :], xT[t][:])

        # ---------- target logit: tl[q] = sum_e xout[e,q] * wte[tgt[q], e] ----------
        with tc.tile_pool(name="tgp", bufs=2) as tgp, \
             tc.tile_pool(name="tlps", bufs=2, space="PSUM") as tlps:
            prod = [tgp.tile([128, T], BF16, tag=f"prod{t}", name=f"prod{t}") for t in range(EC)]
            for t in range(EC):
                tg = tgp.tile([128, T], F32, tag="tg")
                nc.sync.dma_start(tg[:], par["tgtembT"].ap()[t * 128:(t + 1) * 128, :])
                nc.vector.tensor_tensor(out=prod[t][:], in0=xT[t][:], in1=tg[:], op=ALU.mult)
            for q in range(NQ):
                qs = slice(q * TQ, (q + 1) * TQ)
                ps = tlps.tile([1, TQ], F32, tag="tlp")
                for t in range(EC):
                    nc.tensor.matmul(ps[:], ones_col_bf[:], prod[t][:, qs],
                                     start=(t == 0), stop=(t == EC - 1))
                nc.vector.tensor_copy(out=tl_sb[:, qs], in_=ps[:])
        if DEBUG:
            nc.sync.dma_start(par["d_tl"].ap()[:, :], tl_sb[:])

        # ---------- lm_head: logits out + per-token sumexp ----------
        lse_all = losspool.tile([128, 8], F32, tag="lse")
        acc = [losspool.tile([128, NV], F32, tag=f"acc{q}", name=f"acc{q}") for q in range(8)]
        with tc.tile_pool(name="wtp", bufs=2) as wtp, \
             tc.tile_pool(name="lsb", bufs=4) as lsb, \
             tc.tile_pool(name="lps", bufs=4, space="PSUM") as lps:
            for v in range(NV):
                v0 = v * TQ
                nv = 512 if v < NV - 1 else V - v0
                wt = []
                for t in range(EC):
                    w = wtp.tile([128, TQ], BF16, tag=f"wt{t}")
                    nc.sync.dma_start(w[:, 0:nv], par["wteT"].ap()[t * 128:(t + 1) * 128, v0:v0 + nv])
                    wt.append(w)
                for qq in range(8):
                    ps = lps.tile([128, TQ], F32, tag="lp")
                    for t in range(EC):
                        nc.tensor.matmul(ps[:, 0:nv], xout_bf[t][:, qq * 128:(qq + 1) * 128],
                                         wt[t][:, 0:nv], start=(t == 0), stop=(t == EC - 1))
                    ls = lsb.tile([128, TQ], F32, tag="ls")
                    nc.vector.tensor_copy(out=ls[:, 0:nv], in_=ps[:, 0:nv])
                    nc.sync.dma_start(par["logits"].ap()[qq * 128:(qq + 1) * 128, v0:v0 + nv],
                                      ls[:, 0:nv])
                    sc = lsb.tile([128, TQ], BF16, tag="sc")
                    nc.scalar.activation(sc[:, 0:nv], ps[:, 0:nv], AF.Exp, bias=0.0, scale=1.0,
                                         accum_out=acc[qq][:, v:v + 1])
            se = losspool.tile([128, 1], F32, tag="se")
            for qq in range(8):
                nc.vector.reduce_sum(out=se[:], in_=acc[qq][:], axis=mybir.AxisListType.X)
                nc.scalar.activation(lse_all[:, qq:qq + 1], se[:], AF.Ln, bias=0.0, scale=1.0)
            if DEBUG:
                nc.sync.dma_start(par["d_lse"].ap()[:, :], lse_all[:])
            lse_red = losspool.tile([128, 1], F32, tag="lsered")
            nc.vector.reduce_sum(out=lse_red[:], in_=lse_all[:], axis=mybir.AxisListType.X)
            lsum = lps.tile([1, 1], F32, tag="lsum")
            nc.tensor.matmul(lsum[:], ones_col_f32[:], lse_red[:], start=True, stop=True)
            tlsum = losspool.tile([1, 1], F32, tag="tlsum")
            nc.vector.reduce_sum(out=tlsum[:], in_=tl_sb[:], axis=mybir.AxisListType.X)
            lv = losspool.tile([1, 2], F32, tag="lv")
            nc.vector.tensor_copy(out=lv[:, 0:1], in_=lsum[:])
            nc.vector.tensor_copy(out=lv[:, 1:2], in_=tlsum[:])
            nc.sync.dma_start(par["lossv"].ap()[:, :], lv[:])


def _prep_core(inputs, layer, batch):
    f32 = np.float32
    bf16 = ml_dtypes.bfloat16
    wte = _SHARED["wte_f32"]
    tok = np.asarray(inputs[f"t{layer}"])[batch].astype(np.int64)
    nxt = np.asarray(inputs[f"t{layer + 1}"])[batch].astype(np.int64)
    m = {}
    m["xembT"] = np.ascontiguousarray(wte[tok].T)
    m["tgtembT"] = np.ascontiguousarray(wte[nxt].T)
    m["wpeT"] = _SHARED["wpeT"]
    name_map = {"ln1w": "ln1_w", "ln1b": "ln1_b", "ln2w": "ln2_w", "ln2b": "ln2_b"}
    for nm, src in name_map.items():
        m[nm] = np.ascontiguousarray(np.asarray(inputs[src], dtype=f32)[layer].reshape(EC, 128).T)
    for nm in ["attn_w", "proj_w", "fc1_w", "fc2_w"]:
        m[nm] = np.asarray(inputs[nm], dtype=f32)[layer].astype(bf16)
    wqk = np.asarray(inputs["attn_w"], dtype=f32)[layer][:, :2 * E]
    m["attn_wqk_lo"] = (wqk - m["attn_w"][:, :2 * E].astype(f32)).astype(bf16)
    for nm, w in [("attn_b", 3 * E), ("proj_b", E), ("fc1_b", 3 * E), ("fc2_b", E)]:
        m[nm] = np.asarray(inputs[nm], dtype=f32)[layer].reshape(1, w).astype(bf16)
    m["wteT"] = _SHARED["wteT"]
    return m


def kernel(**inputs):
    nc = _build()
    wte = np.asarray(inputs["wte"], dtype=np.float32)
    _SHARED["wte_f32"] = wte
    _SHARED["wteT"] = np.ascontiguousarray(wte.T).astype(ml_dtypes.bfloat16)
    _SHARED["wpeT"] = np.ascontiguousarray(np.asarray(inputs["wpe"], dtype=np.float32)[:T].T)

    in_maps = [_prep_core(inputs, c // 2, c % 2) for c in range(8)]
    res = run_bass_kernel_spmd(nc, in_maps, core_ids=list(range(8)), trace=TRACE)
    LAST_RESULT["exec_time_ns"] = res.exec_time_ns
    LAST_RESULT["results"] = res.results

    logits = np.stack([res.results[6]["logits"], res.results[7]["logits"]], axis=0)
    lsum = np.float64(0.0)
    for c in range(8):
        lv = res.results[c]["lossv"].astype(np.float64)
        lsum += lv[0, 0] - lv[0, 1]
    loss = np.float32(lsum / (B * T))
    return logits, loss
